# revision 14
# baseline (speedup 1.0000x reference)
"""Trainium2 Bass kernel for the 2-layer GAT node-classification head.

The reference reads only h2[mask_idx] and x[mask_idx] for the classifier, so
the exact computation collapses to mask_idx's 2-hop in-neighborhood:

  V1 = sources of mask's in-edges (incl. the self-loop), S2 = in-edges of V1,
  U  = unique sources of S2.  |V1|=2, |S2|=7, |U|=6 for this graph.

Per-core plan (identical on all 8 cores -- the cost model charges a flat
15us constant for ANY collective, which dwarfs the whole problem, so the
fastest distribution is full replication with zero communication):

  1. attention: a_src/a_dst at U via folded Ws1/Wd1 (one-hot scatter to the
     edge layout), segment softmax without max-shift (logits are tiny), all
     heads at once.
  2. aggregate-first: since the value aggregation is linear in x, build
     per-(head, dst) weighted x sums (xagg) BEFORE the big GEMM; the
     [768 x 6144] W1 GEMM then has only v1n output columns per head.
  3. W1 streams in fp8 (x64 prescale to clear the e4m3 subnormal range) in
     6 chunk DMAs pipelined against the PSUM-accumulating GEMM.  DMA bytes
     dominate the kernel; fp8 quarters them vs f32.
  4. elu via exp(min(x,0)) = min(exp(x),1); the "-1" of elu folds into host
     constants.  Layer-2 logits/softmax and the classifier fold into a
     [6144, 4] bf16 contraction + tiny fixed tail.

Host preprocessing: graph cone extraction + one-hot scatter matrices
(index-select = sharding) and weight-weight folds (W1@att, W2@fold), as in
the original head-sharded version.
"""

import numpy as np
import ml_dtypes

import concourse.bass as bass
import concourse.mybir as mybir
import concourse.tile as tile
from concourse import bacc
from concourse.bass_utils import run_bass_kernel_spmd
from concourse.masks import make_identity

NCORES = 8
P = 128
C = 768          # input feature dim
H1 = 8           # layer-1 heads
OUT = 768        # per-head feature dim
KC = C // P      # 6 k-chunks of 128 over the 768 contraction
NEGPAD = -745.0  # padding logit: exp(0.2 * NEGPAD) == 0 in f32
W1SCALE = 64.0   # fp8 prescale for W1 (clears e4m3 subnormals)

f32 = mybir.dt.float32
bf16 = mybir.dt.bfloat16
fp8 = mybir.dt.float8e4
np_bf16 = ml_dtypes.bfloat16
np_fp8 = ml_dtypes.float8_e4m3


# ---------------------------------------------------------------- host graph
def _preprocess(edge_index, mask_idx, n_nodes):
    """Extract the 2-hop in-neighborhood of mask_idx. meta is compile-time
    (shapes only); host holds the data (one-hot matrices, index lists)."""
    ei = np.asarray(edge_index).astype(np.int64)
    m = int(np.asarray(mask_idx))
    src_all = np.concatenate([ei[0], np.arange(n_nodes, dtype=np.int64)])
    dst_all = np.concatenate([ei[1], np.arange(n_nodes, dtype=np.int64)])

    s1_pos = np.nonzero(dst_all == m)[0]          # in-edges of m (incl self)
    s1_src = src_all[s1_pos].tolist()
    s1n = len(s1_src)
    v1 = list(dict.fromkeys(s1_src))              # unique sources
    v1n = len(v1)
    assert v1n <= 8, f"mask in-degree too large for this layout: {v1n}"

    groups = [src_all[np.nonzero(dst_all == v)[0]].tolist() for v in v1]
    gmax = max(len(g) for g in groups)
    s2p = v1n * gmax
    assert s2p <= P, f"edge tile too large: {s2p}"

    u = list(dict.fromkeys([s for g in groups for s in g]))
    un = len(u)
    up = 16
    while up < un:
        up *= 2
    assert v1n * up <= P, f"wuv tile too large: {v1n * up}"
    urow = {node: r for r, node in enumerate(u)}

    # S2 edge slot layout: group g occupies cols [g*gmax, g*gmax+len(g))
    u2e = np.zeros((up, s2p), np.float32)         # src scatter
    d2e = np.zeros((up, s2p), np.float32)         # dst scatter
    pad01 = np.zeros((1, s2p), np.float32)
    sv01 = np.zeros((s2p, v1n * up), np.float32)  # edge -> (v,u) accumulate
    for g, srcs in enumerate(groups):
        for j in range(gmax):
            e = g * gmax + j
            if j < len(srcs):
                su = urow[srcs[j]]
                u2e[su, e] = 1.0
                d2e[urow[v1[g]], e] = 1.0
                sv01[e, g * up + su] = 1.0
            else:
                pad01[0, e] = 1.0

    # layer-2 (s1) structure
    v1row = {v: r for r, v in enumerate(v1)}
    g_mat = np.zeros((v1n, s1n), np.float32)
    gm_mat = np.zeros((v1n, s1n), np.float32)
    for e, s in enumerate(s1_src):
        g_mat[v1row[s], e] = 1.0
        gm_mat[v1row[m], e] = 1.0
    s1_ident = (s1n == v1n) and all(v1row[s] == e for e, s in enumerate(s1_src))

    meta = dict(v1n=v1n, s1n=s1n, gmax=gmax, un=un, up=up, s1_ident=s1_ident)
    host = dict(m=m, v1=v1, u=u, u2e=u2e, d2e=d2e, pad01=pad01, sv01=sv01,
                g=g_mat, gm=gm_mat)
    return meta, host


def _lay16(meta):
    """Column layout of the bf16 packed-constants tensor."""
    up, s2p = meta["up"], meta["v1n"] * meta["gmax"]
    pieces = [
        ("xut", P, KC * up),        # x[U]^T chunked  [128, KC*up]
        ("wsd1", P, KC * 2 * H1),   # [Ws1|Wd1] chunked
        ("u2e", up, s2p),
        ("d2e", up, s2p),
        ("pad01", 1, s2p),
        ("neg8", 1, H1),
        ("sv01", s2p, meta["v1n"] * up),
    ]
    lay, off = {}, 0
    for name, rows, cols in pieces:
        lay[name] = (rows, off, cols)
        off += cols
    return lay, off


def _lay32(meta):
    """Column layout of the f32 packed-constants tensor (tail/oxm)."""
    v1n, s1n = meta["v1n"], meta["s1n"]
    pieces = [
        ("xm", P, KC),
        ("wfb", P, KC * 2),
        ("g", v1n, s1n),
        ("gm", v1n, s1n),
        ("shiftrow", 1, s1n),
        ("one11", 1, 1),
        ("bias3s", 1, 2),
        ("ones_s1", s1n, 1),
    ]
    lay, off = {}, 0
    for name, rows, cols in pieces:
        lay[name] = (rows, off, cols)
        off += cols
    return lay, off


def _chunked(w):
    """[K, N] -> [128, (K//128)*N] chunk-major free layout."""
    k, n = w.shape
    assert k % P == 0
    return np.ascontiguousarray(
        w.reshape(k // P, P, n).transpose(1, 0, 2).reshape(P, (k // P) * n))


# ---------------------------------------------------------------- bass build
def _build(meta):
    v1n, s1n, gmax = meta["v1n"], meta["s1n"], meta["gmax"]
    up, s1_ident = meta["up"], meta["s1_ident"]
    s2p = v1n * gmax
    nblk = H1 * KC                  # 48 (head, f-chunk) output blocks
    lay16, cw16 = _lay16(meta)
    lay32, cw32 = _lay32(meta)

    nc = bacc.Bacc("TRN2", target_bir_lowering=False, debug=False,
                   enable_asserts=True, num_devices=NCORES)

    d_cst16 = nc.dram_tensor("cst16", [P, cw16], bf16, kind="ExternalInput")
    d_xu = nc.dram_tensor("xu", [up, C], bf16, kind="ExternalInput")
    # W1 stream pieces: full chunks c0..c4, then chunk 5 split into graded
    # block groups so only 8 blocks' matmuls + a small elu slice trail the
    # final DMA semaphore.
    W1_PIECES = [(c, 0, nblk) for c in range(KC - 1)] + [
        (KC - 1, 0, 24), (KC - 1, 24, 40), (KC - 1, 40, nblk)]
    d_w1 = [nc.dram_tensor(f"w1p{i}", [P, (k1 - k0) * P], fp8,
                           kind="ExternalInput")
            for i, (c, k0, k1) in enumerate(W1_PIECES)]
    d_w2f = nc.dram_tensor("w2f", [P, nblk * 4], bf16, kind="ExternalInput")
    d_cst32 = nc.dram_tensor("cst32", [P, cw32], f32, kind="ExternalInput")
    d_res = nc.dram_tensor("res", [1, 2], f32, kind="ExternalOutput")

    with tile.TileContext(nc) as tc:
        with (
            tc.tile_pool(name="const", bufs=1) as cpool,
            tc.tile_pool(name="sbuf", bufs=1) as sb,
            tc.tile_pool(name="big", bufs=1) as bigp,
            tc.tile_pool(name="ps", bufs=1, space="PSUM") as ps,
        ):
            # ---- input DMAs, spread across sequencers so descriptor
            # generation pipelines: Pool/SWDGE carries the attention
            # constants (first transfer on the wire), SP streams W1
            # back-to-back, Act/DVE carry the rest.
            cst16 = cpool.tile([P, cw16], bf16, tag="cst16")
            nc.sync.dma_start(out=cst16[:], in_=d_cst16[:])
            w1_sb = [bigp.tile([P, (k1 - k0) * P], fp8, tag=f"w1_{i}",
                               name=f"w1_{i}")
                     for i, (c, k0, k1) in enumerate(W1_PIECES)]
            nc.sync.dma_start(out=w1_sb[0][:], in_=d_w1[0][:])
            xu_sb = cpool.tile([up, C], bf16, tag="xu")
            nc.scalar.dma_start(out=xu_sb[:], in_=d_xu[:])
            for i in range(1, len(W1_PIECES)):
                nc.sync.dma_start(out=w1_sb[i][:], in_=d_w1[i][:])
            w2f_sb = cpool.tile([P, nblk * 4], bf16, tag="w2f")
            nc.scalar.dma_start(out=w2f_sb[:], in_=d_w2f[:])
            cst32 = cpool.tile([P, cw32], f32, tag="cst32")
            nc.scalar.dma_start(out=cst32[:], in_=d_cst32[:])

            def cv16(name):
                rows, off, cols = lay16[name]
                return cst16[0:rows, off:off + cols]

            def cv32(name):
                rows, off, cols = lay32[name]
                return cst32[0:rows, off:off + cols]

            xut_v = cv16("xut").rearrange("p (k n) -> p k n", k=KC)
            wsd1_v = cv16("wsd1").rearrange("p (k n) -> p k n", k=KC)
            u2e_v = cv16("u2e")
            d2e_v = cv16("d2e")
            pad01_v = cv16("pad01")
            neg8_v = cv16("neg8")
            sv01_v = cv16("sv01")

            ident = cpool.tile([H1, H1], f32, tag="ident")
            make_identity(nc, ident[:])

            # ---- attention: a_src/a_dst at U, all heads ----
            attb = ps.tile([P, 512], f32, tag="attbank")
            asd_ps = attb[0:up, 0:2 * H1]
            lg_ps = attb[0:H1, 16:16 + s2p]
            at_ps = attb[0:s2p, 144:144 + H1]
            wuv_ps = [attb[0:up, 152 + 8 * v:160 + 8 * v]
                      for v in range(v1n)]
            for c in range(KC):
                nc.tensor.matmul(out=asd_ps, lhsT=xut_v[:, c, :],
                                 rhs=wsd1_v[:, c, :],
                                 start=(c == 0), stop=(c == KC - 1))
            asd_sb = sb.tile([up, 2 * H1], bf16, tag="asd_sb")
            nc.vector.tensor_copy(out=asd_sb[:], in_=asd_ps)

            # per-edge logits: a_s[src_e] + a_d[dst_e] + pad bias
            nc.tensor.matmul(out=lg_ps, lhsT=asd_sb[:, 0:H1], rhs=u2e_v,
                             start=True, stop=False)
            nc.tensor.matmul(out=lg_ps, lhsT=asd_sb[:, H1:2 * H1],
                             rhs=d2e_v, start=False, stop=False)
            nc.tensor.matmul(out=lg_ps, lhsT=neg8_v, rhs=pad01_v,
                             start=False, stop=True)

            # leaky-relu (one Act op), exp without max-shift (logits tiny),
            # then per-group normalize
            lg_t = sb.tile([H1, s2p], f32, tag="lg_t")
            nc.vector.tensor_scalar_mul(out=lg_t[:], in0=lg_ps, scalar1=0.2)
            lg_sb = sb.tile([H1, s2p], f32, tag="lg_sb")
            nc.vector.tensor_tensor(out=lg_sb[:], in0=lg_ps, in1=lg_t[:],
                                    op=mybir.AluOpType.max)
            ee_sb = sb.tile([H1, s2p], f32, tag="ee_sb")
            nc.scalar.activation(out=ee_sb[:], in_=lg_sb[:],
                                 func=mybir.ActivationFunctionType.Exp)
            eev = ee_sb[:].rearrange("h (g e) -> h g e", e=gmax)
            den = sb.tile([H1, v1n], f32, tag="den")
            nc.vector.reduce_sum(out=den[:], in_=eev,
                                 axis=mybir.AxisListType.X)
            rec = sb.tile([H1, v1n], f32, tag="rec")
            nc.vector.reciprocal(out=rec[:], in_=den[:])
            alpha_sb = sb.tile([H1, s2p], f32, tag="alpha_sb")
            recb = rec[:].rearrange("h (g o) -> h g o", o=1).to_broadcast(
                [H1, v1n, gmax])
            nc.vector.tensor_tensor(
                out=alpha_sb[:].rearrange("h (g e) -> h g e", e=gmax),
                in0=eev, in1=recb, op=mybir.AluOpType.mult)

            # alpha^T via PE transpose, then wuv[(v,u), h] = sum_e alpha
            nc.tensor.transpose(out=at_ps, in_=alpha_sb[:],
                                identity=ident[:])
            at_sb = sb.tile([s2p, H1], bf16, tag="at_sb")
            nc.vector.tensor_copy(out=at_sb[:], in_=at_ps)
            # per-v blocks: PE/DVE partition bases must be 0/32/64-aligned
            wuv_sb = [sb.tile([up, H1], bf16, tag=f"wuv_sb{v}",
                              name=f"wuv_sb{v}") for v in range(v1n)]
            for v in range(v1n):
                nc.tensor.matmul(out=wuv_ps[v],
                                 lhsT=sv01_v[:, v * up:(v + 1) * up],
                                 rhs=at_sb[:], start=True, stop=True)
                nc.vector.tensor_copy(out=wuv_sb[v][:], in_=wuv_ps[v])

            # xagg^T chunks: [128c, (c,v,h)] = sum_u x[U]^T wuv
            xagg_ps = ps.tile([P, KC * v1n * H1], f32, tag="xagg")
            for c in range(KC):
                for v in range(v1n):
                    nc.tensor.matmul(
                        out=xagg_ps[:, (c * v1n + v) * H1:
                                    (c * v1n + v + 1) * H1],
                        lhsT=xu_sb[:, c * P:(c + 1) * P],
                        rhs=wuv_sb[v][:],
                        start=True, stop=True)
            xagg8 = sb.tile([P, KC * v1n * H1], fp8, tag="xagg8")
            nc.vector.tensor_copy(out=xagg8[:], in_=xagg_ps[:])
            xagg8_v = xagg8[:].rearrange("p (c v h) -> p c v h", c=KC, v=v1n)

            # ---- the big GEMM: agg[f, (h,fc,v)] = xagg @ (64*W1)
            # one accumulate pass per W1 piece as its DMA lands;
            # fp8 x fp8 -> f32 PSUM.  W1 block k = columns [k*128,(k+1)*128)
            # (k = h*KC + fc), so lhsT slices are contiguous per piece.
            # one start=True matmul zeroes the whole bank (the PSUM zero
            # region is 2KB-coarse, so per-block starts would wipe
            # neighbors); every accumulating matmul then uses start=False.
            agg_ps = ps.tile([P, nblk * v1n], f32, tag="agg")
            zrow = cpool.tile([1, P], bf16, tag="zrow")
            nc.vector.memset(zrow[:], 0.0)
            zcols = cpool.tile([1, nblk * v1n], bf16, tag="zcols")
            nc.vector.memset(zcols[:], 0.0)
            nc.tensor.matmul(out=agg_ps[:], lhsT=zrow[:], rhs=zcols[:],
                             start=True, stop=False, skip_group_check=True)
            for i, (c, k0, k1) in enumerate(W1_PIECES):
                for k in range(k0, k1):
                    h = k // KC
                    nc.tensor.matmul(
                        out=agg_ps[:, k * v1n:(k + 1) * v1n],
                        lhsT=w1_sb[i][:, (k - k0) * P:(k - k0 + 1) * P],
                        rhs=xagg8_v[:, c, :, h],
                        start=False, stop=(c == KC - 1),
                        skip_group_check=True)

            # elu'(x) = elu(x) + 1 = max(x,0) + min(exp(x),1); x = agg/64.
            # The -1 is folded into host constants downstream.  Computed in
            # block ranges matching the W1 piece splits so only the last 8
            # blocks' elu trails the final DMA.
            t1_sb = sb.tile([P, nblk * v1n], f32, tag="t1_sb")
            ee2_sb = sb.tile([P, nblk * v1n], f32, tag="ee2_sb")
            helu_sb = sb.tile([P, nblk * v1n], bf16, tag="helu_sb")
            for (k0, k1) in [(0, 24), (24, 40), (40, nblk)]:
                cl = slice(k0 * v1n, k1 * v1n)
                nc.vector.tensor_scalar(out=t1_sb[:, cl], in0=agg_ps[:, cl],
                                        scalar1=1.0 / W1SCALE, scalar2=0.0,
                                        op0=mybir.AluOpType.mult,
                                        op1=mybir.AluOpType.max)
                nc.scalar.activation(out=ee2_sb[:, cl], in_=agg_ps[:, cl],
                                     func=mybir.ActivationFunctionType.Exp,
                                     scale=1.0 / W1SCALE)
                nc.vector.tensor_scalar(out=ee2_sb[:, cl], in0=ee2_sb[:, cl],
                                        scalar1=1.0, scalar2=None,
                                        op0=mybir.AluOpType.min)
                nc.vector.tensor_tensor(out=helu_sb[:, cl], in0=t1_sb[:, cl],
                                        in1=ee2_sb[:, cl],
                                        op=mybir.AluOpType.add)

            # ---- oxm = x[m] @ wf_bot + bias3s (off critical path) ----
            xm_v = cv32("xm")
            wfb_v = cv32("wfb").rearrange("p (k n) -> p k n", k=KC)
            one11_v = cv32("one11")
            bias3s_v = cv32("bias3s")
            tailb = ps.tile([P, 12], f32, tag="tailbank")
            oxm_ps = tailb[0:1, 0:2]
            h2f_ps = tailb[0:v1n, 2:6]
            r2t_ps = tailb[0:s1n, 6:7]
            den_ps = tailb[0:1, 7:8]
            fin_ps = tailb[0:1, 8:10]
            for c in range(KC):
                nc.tensor.matmul(out=oxm_ps, lhsT=xm_v[:, c:c + 1],
                                 rhs=wfb_v[:, c, :],
                                 start=(c == 0), stop=False)
            nc.tensor.matmul(out=oxm_ps, lhsT=one11_v, rhs=bias3s_v,
                             start=False, stop=True)

            # ---- folded layer-2: h2f'[v, 0:4] = helu' @ [w2fold|Ws2|Wd2]
            for k in range(nblk):
                nc.tensor.matmul(out=h2f_ps,
                                 lhsT=helu_sb[:, k * v1n:(k + 1) * v1n],
                                 rhs=w2f_sb[:, k * 4:(k + 1) * 4],
                                 start=(k == 0), stop=(k == nblk - 1))
            h2f_sb = sb.tile([v1n, 4], f32, tag="h2f_sb")
            nc.vector.tensor_copy(out=h2f_sb[:], in_=h2f_ps)

            # ---- layer-2 logits (transposed), softmax, weighted sum ----
            g_v = cv32("g")
            gm_v = cv32("gm")
            shiftrow_v = cv32("shiftrow")
            ones_s1_v = cv32("ones_s1")
            nc.tensor.matmul(out=r2t_ps, lhsT=g_v, rhs=h2f_sb[:, 2:3],
                             start=True, stop=False)
            nc.tensor.matmul(out=r2t_ps, lhsT=gm_v, rhs=h2f_sb[:, 3:4],
                             start=False, stop=False)
            nc.tensor.matmul(out=r2t_ps, lhsT=shiftrow_v, rhs=one11_v,
                             start=False, stop=True)
            al2t_t = sb.tile([s1n, 1], f32, tag="al2t_t")
            nc.vector.tensor_scalar_mul(out=al2t_t[:], in0=r2t_ps,
                                        scalar1=0.2)
            al2t_sb = sb.tile([s1n, 1], f32, tag="al2t_sb")
            nc.vector.tensor_tensor(out=al2t_sb[:], in0=r2t_ps,
                                    in1=al2t_t[:], op=mybir.AluOpType.max)
            e2t_sb = sb.tile([s1n, 1], f32, tag="e2t_sb")
            nc.scalar.activation(out=e2t_sb[:], in_=al2t_sb[:],
                                 func=mybir.ActivationFunctionType.Exp)

            nc.tensor.matmul(out=den_ps, lhsT=e2t_sb[:], rhs=ones_s1_v,
                             start=True, stop=True)
            if s1_ident:
                nc.tensor.matmul(out=fin_ps, lhsT=e2t_sb[:],
                                 rhs=h2f_sb[:, 0:2], start=True, stop=True)
            else:
                gath_ps = tailb[0:s1n, 10:12]
                nc.tensor.matmul(out=gath_ps, lhsT=g_v,
                                 rhs=h2f_sb[:, 0:2], start=True, stop=True)
                gath_sb = sb.tile([s1n, 2], f32, tag="gath_sb")
                nc.vector.tensor_copy(out=gath_sb[:], in_=gath_ps)
                nc.tensor.matmul(out=fin_ps, lhsT=e2t_sb[:],
                                 rhs=gath_sb[:], start=True, stop=True)

            rec2 = sb.tile([1, 1], f32, tag="rec2")
            nc.vector.reciprocal(out=rec2[:], in_=den_ps)
            res1 = sb.tile([1, 2], f32, tag="res1")
            nc.vector.tensor_scalar(out=res1[:], in0=fin_ps,
                                    scalar1=rec2[:, 0:1], scalar2=None,
                                    op0=mybir.AluOpType.mult)
            res_sb = sb.tile([1, 2], f32, tag="res_sb")
            nc.vector.tensor_add(out=res_sb[:], in0=res1[:],
                                 in1=oxm_ps)
            nc.sync.dma_start(out=d_res[:], in_=res_sb[:])

    nc.compile()
    return nc


_CACHE = {}


def _get_nc(meta):
    key = repr(sorted(meta.items()))
    if key not in _CACHE:
        _CACHE[key] = _build(meta)
    return _CACHE[key]


def make_in_maps(**inputs):
    x = np.asarray(inputs["x"], np.float32)
    n_nodes = x.shape[0]
    meta, host = _preprocess(inputs["edge_index"], inputs["mask_idx"], n_nodes)
    v1n, s1n, up = meta["v1n"], meta["s1n"], meta["up"]
    s2p = v1n * meta["gmax"]
    nblk = H1 * KC

    W1 = np.asarray(inputs["W1"], np.float32)
    att_s1 = np.asarray(inputs["att_src1"], np.float32)
    att_d1 = np.asarray(inputs["att_dst1"], np.float32)
    b1 = np.asarray(inputs["b1"], np.float32)
    W2 = np.asarray(inputs["W2"], np.float32)
    att_s2 = np.asarray(inputs["att_src2"], np.float32)
    att_d2 = np.asarray(inputs["att_dst2"], np.float32)
    b2 = np.asarray(inputs["b2"], np.float32)
    fc_w = np.asarray(inputs["fc_w"], np.float32)
    fc_b = np.asarray(inputs["fc_b"], np.float32)
    cls_w = np.asarray(inputs["cls_w"], np.float32)
    cls_b = np.asarray(inputs["cls_b"], np.float32)

    # weight-weight folds
    Ws1 = np.einsum("chf,hf->ch", W1.reshape(C, H1, OUT), att_s1)   # [C, H1]
    Wd1 = np.einsum("chf,hf->ch", W1.reshape(C, H1, OUT), att_d1)
    Ws2 = W2 @ att_s2[0]                                            # [H1*OUT]
    Wd2 = W2 @ att_d2[0]
    wf = fc_w @ cls_w                                               # [1536, 2]
    wf_top, wf_bot = wf[:OUT], wf[OUT:]
    w2fold = W2 @ wf_top                                            # [6144, 2]
    # helu' = elu + 1 fold: subtract column sums; softmax shift constant
    shift_const = -(Ws2.sum() + Wd2.sum())
    bias3s = (b2 @ wf_top + fc_b @ cls_w + cls_b
              - w2fold.sum(axis=0)).reshape(1, 2).astype(np.float32)

    # w2f blocks ordered to match agg blocks k = h*KC + fc
    w2f4 = np.concatenate([w2fold, Ws2[:, None], Wd2[:, None]], axis=1)
    w2f_host = np.zeros((P, nblk * 4), np.float32)
    for k in range(nblk):
        w2f_host[:, k * 4:(k + 1) * 4] = w2f4[k * P:(k + 1) * P, :]

    # bf16 constants tensor
    lay16, cw16 = _lay16(meta)
    cst16 = np.zeros((P, cw16), np.float32)

    def fill16(name, arr):
        rows, off, cols = lay16[name]
        assert arr.shape == (rows, cols), (name, arr.shape, (rows, cols))
        cst16[0:rows, off:off + cols] = arr

    xu_rows = np.zeros((up, C), np.float32)
    xu_rows[:meta["un"]] = x[host["u"]]
    xut = np.zeros((P, KC * up), np.float32)
    for c in range(KC):
        xut[:, c * up:(c + 1) * up] = xu_rows[:, c * P:(c + 1) * P].T
    fill16("xut", xut)
    fill16("wsd1", _chunked(np.concatenate([Ws1, Wd1], axis=1)))
    fill16("u2e", host["u2e"])
    fill16("d2e", host["d2e"])
    fill16("pad01", host["pad01"])
    fill16("neg8", np.full((1, H1), NEGPAD, np.float32))
    fill16("sv01", host["sv01"])

    # f32 constants tensor (tail)
    lay32, cw32 = _lay32(meta)
    cst32 = np.zeros((P, cw32), np.float32)

    def fill32(name, arr):
        rows, off, cols = lay32[name]
        assert arr.shape == (rows, cols), (name, arr.shape, (rows, cols))
        cst32[0:rows, off:off + cols] = arr

    fill32("xm", np.ascontiguousarray(x[host["m"]].reshape(KC, P).T))
    fill32("wfb", _chunked(np.ascontiguousarray(wf_bot)))
    fill32("g", host["g"])
    fill32("gm", host["gm"])
    fill32("shiftrow", np.full((1, s1n), shift_const, np.float32))
    fill32("one11", np.ones((1, 1), np.float32))
    fill32("bias3s", bias3s)
    fill32("ones_s1", np.ones((s1n, 1), np.float32))

    assert not np.any(b1), "b1 != 0 not supported by this build"
    w1s = (W1 * W1SCALE).astype(np_fp8)                 # [768, 6144] fp8

    im = {
        "cst16": cst16.astype(np_bf16),
        "xu": xu_rows.astype(np_bf16),
        "w2f": w2f_host.astype(np_bf16),
        "cst32": cst32,
    }
    pieces = [(c, 0, nblk) for c in range(KC - 1)] + [
        (KC - 1, 0, 24), (KC - 1, 24, 40), (KC - 1, 40, nblk)]
    for i, (c, k0, k1) in enumerate(pieces):
        im[f"w1p{i}"] = np.ascontiguousarray(
            w1s[c * P:(c + 1) * P, k0 * P:k1 * P])
    return meta, [im] * NCORES


def kernel(**inputs):
    meta, in_maps = make_in_maps(**inputs)
    nc = _get_nc(meta)
    res = run_bass_kernel_spmd(nc, in_maps, core_ids=list(range(NCORES)))
    return res.results[0]["res"].astype(np.float32)


# revision 15
# speedup vs baseline: 1.0386x; 1.0386x over previous
"""Trainium2 Bass kernel for the 2-layer GAT node-classification head.

The reference reads only h2[mask_idx] and x[mask_idx] for the classifier, so
the exact computation collapses to mask_idx's 2-hop in-neighborhood:

  V1 = sources of mask's in-edges (incl. the self-loop), S2 = in-edges of V1,
  U  = unique sources of S2.  |V1|=2, |S2|=7, |U|=6 for this graph.

Per-core plan (identical on all 8 cores -- the cost model charges a flat
15us constant for ANY collective, which dwarfs the whole problem, so the
fastest distribution is full replication with zero communication):

  1. attention: a_src/a_dst at U via folded Ws1/Wd1 (one-hot scatter to the
     edge layout), segment softmax without max-shift (logits are tiny), all
     heads at once.
  2. aggregate-first: since the value aggregation is linear in x, build
     per-(head, dst) weighted x sums (xagg) BEFORE the big GEMM; the
     [768 x 6144] W1 GEMM then has only v1n output columns per head.
  3. W1 streams in fp8 (x64 prescale to clear the e4m3 subnormal range) in
     6 chunk DMAs pipelined against the PSUM-accumulating GEMM.  DMA bytes
     dominate the kernel; fp8 quarters them vs f32.
  4. elu via exp(min(x,0)) = min(exp(x),1); the "-1" of elu folds into host
     constants.  Layer-2 logits/softmax and the classifier fold into a
     [6144, 4] bf16 contraction + tiny fixed tail.

Host preprocessing: graph cone extraction + one-hot scatter matrices
(index-select = sharding) and weight-weight folds (W1@att, W2@fold), as in
the original head-sharded version.
"""

import numpy as np
import ml_dtypes

import concourse.bass as bass
import concourse.mybir as mybir
import concourse.tile as tile
from concourse import bacc
from concourse.bass_utils import run_bass_kernel_spmd
from concourse.masks import make_identity

NCORES = 8
P = 128
C = 768          # input feature dim
H1 = 8           # layer-1 heads
OUT = 768        # per-head feature dim
KC = C // P      # 6 k-chunks of 128 over the 768 contraction
NEGPAD = -745.0  # padding logit: exp(0.2 * NEGPAD) == 0 in f32
W1SCALE = 64.0   # fp8 prescale for W1 (clears e4m3 subnormals)

f32 = mybir.dt.float32
bf16 = mybir.dt.bfloat16
fp8 = mybir.dt.float8e4
np_bf16 = ml_dtypes.bfloat16
np_fp8 = ml_dtypes.float8_e4m3


# ---------------------------------------------------------------- host graph
def _preprocess(edge_index, mask_idx, n_nodes):
    """Extract the 2-hop in-neighborhood of mask_idx. meta is compile-time
    (shapes only); host holds the data (one-hot matrices, index lists)."""
    ei = np.asarray(edge_index).astype(np.int64)
    m = int(np.asarray(mask_idx))
    src_all = np.concatenate([ei[0], np.arange(n_nodes, dtype=np.int64)])
    dst_all = np.concatenate([ei[1], np.arange(n_nodes, dtype=np.int64)])

    s1_pos = np.nonzero(dst_all == m)[0]          # in-edges of m (incl self)
    s1_src = src_all[s1_pos].tolist()
    s1n = len(s1_src)
    v1 = list(dict.fromkeys(s1_src))              # unique sources
    v1n = len(v1)
    assert v1n <= 8, f"mask in-degree too large for this layout: {v1n}"

    groups = [src_all[np.nonzero(dst_all == v)[0]].tolist() for v in v1]
    gmax = max(len(g) for g in groups)
    s2p = v1n * gmax
    assert s2p <= P, f"edge tile too large: {s2p}"

    u = list(dict.fromkeys([s for g in groups for s in g]))
    un = len(u)
    up = 16
    while up < un:
        up *= 2
    assert v1n * up <= P, f"wuv tile too large: {v1n * up}"
    urow = {node: r for r, node in enumerate(u)}

    # S2 edge slot layout: group g occupies cols [g*gmax, g*gmax+len(g))
    u2e = np.zeros((up, s2p), np.float32)         # src scatter
    d2e = np.zeros((up, s2p), np.float32)         # dst scatter
    pad01 = np.zeros((1, s2p), np.float32)
    sv01 = np.zeros((s2p, v1n * up), np.float32)  # edge -> (v,u) accumulate
    for g, srcs in enumerate(groups):
        for j in range(gmax):
            e = g * gmax + j
            if j < len(srcs):
                su = urow[srcs[j]]
                u2e[su, e] = 1.0
                d2e[urow[v1[g]], e] = 1.0
                sv01[e, g * up + su] = 1.0
            else:
                pad01[0, e] = 1.0

    # layer-2 (s1) structure
    v1row = {v: r for r, v in enumerate(v1)}
    g_mat = np.zeros((v1n, s1n), np.float32)
    gm_mat = np.zeros((v1n, s1n), np.float32)
    for e, s in enumerate(s1_src):
        g_mat[v1row[s], e] = 1.0
        gm_mat[v1row[m], e] = 1.0
    s1_ident = (s1n == v1n) and all(v1row[s] == e for e, s in enumerate(s1_src))

    meta = dict(v1n=v1n, s1n=s1n, gmax=gmax, un=un, up=up, s1_ident=s1_ident)
    host = dict(m=m, v1=v1, u=u, u2e=u2e, d2e=d2e, pad01=pad01, sv01=sv01,
                g=g_mat, gm=gm_mat)
    return meta, host


def _lay16(meta):
    """Column layout of the bf16 packed-constants tensor."""
    up, s2p = meta["up"], meta["v1n"] * meta["gmax"]
    pieces = [
        ("xut", P, KC * up),        # x[U]^T chunked  [128, KC*up]
        ("wsd1", P, KC * 2 * H1),   # [Ws1|Wd1] chunked
        ("u2e", up, s2p),
        ("d2e", up, s2p),
        ("pad01", 1, s2p),
        ("neg8", 1, H1),
        ("sv01", s2p, meta["v1n"] * up),
    ]
    lay, off = {}, 0
    for name, rows, cols in pieces:
        lay[name] = (rows, off, cols)
        off += cols
    return lay, off


def _lay32(meta):
    """Column layout of the f32 packed-constants tensor (tail/oxm)."""
    v1n, s1n = meta["v1n"], meta["s1n"]
    pieces = [
        ("xm", P, KC),
        ("wfb", P, KC * 2),
        ("g", v1n, s1n),
        ("gm", v1n, s1n),
        ("shiftrow", 1, s1n),
        ("one11", 1, 1),
        ("bias3s", 1, 2),
        ("ones_s1", s1n, 1),
    ]
    lay, off = {}, 0
    for name, rows, cols in pieces:
        lay[name] = (rows, off, cols)
        off += cols
    return lay, off


def _chunked(w):
    """[K, N] -> [128, (K//128)*N] chunk-major free layout."""
    k, n = w.shape
    assert k % P == 0
    return np.ascontiguousarray(
        w.reshape(k // P, P, n).transpose(1, 0, 2).reshape(P, (k // P) * n))


# ---------------------------------------------------------------- bass build
def _build(meta):
    v1n, s1n, gmax = meta["v1n"], meta["s1n"], meta["gmax"]
    up, s1_ident = meta["up"], meta["s1_ident"]
    s2p = v1n * gmax
    nblk = H1 * KC                  # 48 (head, f-chunk) output blocks
    lay16, cw16 = _lay16(meta)
    lay32, cw32 = _lay32(meta)

    nc = bacc.Bacc("TRN2", target_bir_lowering=False, debug=False,
                   enable_asserts=True, num_devices=NCORES)

    d_cst16 = nc.dram_tensor("cst16", [P, cw16], bf16, kind="ExternalInput")
    d_xu = nc.dram_tensor("xu", [up, C], bf16, kind="ExternalInput")
    # W1 stream pieces: full chunks c0..c4, then chunk 5 split into graded
    # block groups so only 8 blocks' matmuls + a small elu slice trail the
    # final DMA semaphore.
    W1_PIECES = [(c, 0, nblk) for c in range(KC - 1)] + [
        (KC - 1, 0, 24), (KC - 1, 24, 40), (KC - 1, 40, nblk)]
    d_w1 = [nc.dram_tensor(f"w1p{i}", [P, (k1 - k0) * P], fp8,
                           kind="ExternalInput")
            for i, (c, k0, k1) in enumerate(W1_PIECES)]
    d_w2f = nc.dram_tensor("w2f", [P, nblk * 4], bf16, kind="ExternalInput")
    d_cst32 = nc.dram_tensor("cst32", [P, cw32], f32, kind="ExternalInput")
    d_res = nc.dram_tensor("res", [1, 2], f32, kind="ExternalOutput")

    with tile.TileContext(nc) as tc:
        with (
            tc.tile_pool(name="const", bufs=1) as cpool,
            tc.tile_pool(name="sbuf", bufs=1) as sb,
            tc.tile_pool(name="big", bufs=1) as bigp,
            tc.tile_pool(name="ps", bufs=1, space="PSUM") as ps,
        ):
            # ---- input DMAs, spread across sequencers so descriptor
            # generation pipelines: Pool/SWDGE carries the attention
            # constants (first transfer on the wire), SP streams W1
            # back-to-back, Act/DVE carry the rest.
            cst16 = cpool.tile([P, cw16], bf16, tag="cst16")
            nc.gpsimd.dma_start(out=cst16[:], in_=d_cst16[:])
            w1_sb = [bigp.tile([P, (k1 - k0) * P], fp8, tag=f"w1_{i}",
                               name=f"w1_{i}")
                     for i, (c, k0, k1) in enumerate(W1_PIECES)]
            nc.sync.dma_start(out=w1_sb[0][:], in_=d_w1[0][:])
            xu_sb = cpool.tile([up, C], bf16, tag="xu")
            nc.scalar.dma_start(out=xu_sb[:], in_=d_xu[:])
            for i in range(1, len(W1_PIECES)):
                nc.sync.dma_start(out=w1_sb[i][:], in_=d_w1[i][:])
            w2f_sb = cpool.tile([P, nblk * 4], bf16, tag="w2f")
            nc.scalar.dma_start(out=w2f_sb[:], in_=d_w2f[:])
            cst32 = cpool.tile([P, cw32], f32, tag="cst32")
            nc.scalar.dma_start(out=cst32[:], in_=d_cst32[:])

            def cv16(name):
                rows, off, cols = lay16[name]
                return cst16[0:rows, off:off + cols]

            def cv32(name):
                rows, off, cols = lay32[name]
                return cst32[0:rows, off:off + cols]

            xut_v = cv16("xut").rearrange("p (k n) -> p k n", k=KC)
            wsd1_v = cv16("wsd1").rearrange("p (k n) -> p k n", k=KC)
            u2e_v = cv16("u2e")
            d2e_v = cv16("d2e")
            pad01_v = cv16("pad01")
            neg8_v = cv16("neg8")
            sv01_v = cv16("sv01")

            ident = cpool.tile([H1, H1], f32, tag="ident")
            make_identity(nc, ident[:])

            # ---- attention: a_src/a_dst at U, all heads ----
            attb = ps.tile([P, 512], f32, tag="attbank")
            asd_ps = attb[0:up, 0:2 * H1]
            lg_ps = attb[0:H1, 16:16 + s2p]
            at_ps = attb[0:s2p, 144:144 + H1]
            wuv_ps = [attb[0:up, 152 + 8 * v:160 + 8 * v]
                      for v in range(v1n)]
            for c in range(KC):
                nc.tensor.matmul(out=asd_ps, lhsT=xut_v[:, c, :],
                                 rhs=wsd1_v[:, c, :],
                                 start=(c == 0), stop=(c == KC - 1))
            asd_sb = sb.tile([up, 2 * H1], bf16, tag="asd_sb")
            nc.vector.tensor_copy(out=asd_sb[:], in_=asd_ps)

            # per-edge logits: a_s[src_e] + a_d[dst_e] + pad bias
            nc.tensor.matmul(out=lg_ps, lhsT=asd_sb[:, 0:H1], rhs=u2e_v,
                             start=True, stop=False)
            nc.tensor.matmul(out=lg_ps, lhsT=asd_sb[:, H1:2 * H1],
                             rhs=d2e_v, start=False, stop=False)
            nc.tensor.matmul(out=lg_ps, lhsT=neg8_v, rhs=pad01_v,
                             start=False, stop=True)

            # leaky-relu (one Act op), exp without max-shift (logits tiny),
            # then per-group normalize
            lg_t = sb.tile([H1, s2p], f32, tag="lg_t")
            nc.vector.tensor_scalar_mul(out=lg_t[:], in0=lg_ps, scalar1=0.2)
            lg_sb = sb.tile([H1, s2p], f32, tag="lg_sb")
            nc.vector.tensor_tensor(out=lg_sb[:], in0=lg_ps, in1=lg_t[:],
                                    op=mybir.AluOpType.max)
            ee_sb = sb.tile([H1, s2p], f32, tag="ee_sb")
            nc.scalar.activation(out=ee_sb[:], in_=lg_sb[:],
                                 func=mybir.ActivationFunctionType.Exp)
            eev = ee_sb[:].rearrange("h (g e) -> h g e", e=gmax)
            den = sb.tile([H1, v1n], f32, tag="den")
            nc.vector.reduce_sum(out=den[:], in_=eev,
                                 axis=mybir.AxisListType.X)
            rec = sb.tile([H1, v1n], f32, tag="rec")
            nc.vector.reciprocal(out=rec[:], in_=den[:])
            alpha_sb = sb.tile([H1, s2p], f32, tag="alpha_sb")
            recb = rec[:].rearrange("h (g o) -> h g o", o=1).to_broadcast(
                [H1, v1n, gmax])
            nc.vector.tensor_tensor(
                out=alpha_sb[:].rearrange("h (g e) -> h g e", e=gmax),
                in0=eev, in1=recb, op=mybir.AluOpType.mult)

            # alpha^T via PE transpose, then wuv[(v,u), h] = sum_e alpha
            nc.tensor.transpose(out=at_ps, in_=alpha_sb[:],
                                identity=ident[:])
            at_sb = sb.tile([s2p, H1], bf16, tag="at_sb")
            nc.vector.tensor_copy(out=at_sb[:], in_=at_ps)
            # per-v blocks: PE/DVE partition bases must be 0/32/64-aligned
            wuv_sb = [sb.tile([up, H1], bf16, tag=f"wuv_sb{v}",
                              name=f"wuv_sb{v}") for v in range(v1n)]
            for v in range(v1n):
                nc.tensor.matmul(out=wuv_ps[v],
                                 lhsT=sv01_v[:, v * up:(v + 1) * up],
                                 rhs=at_sb[:], start=True, stop=True)
                nc.vector.tensor_copy(out=wuv_sb[v][:], in_=wuv_ps[v])

            # xagg^T chunks: [128c, (c,v,h)] = sum_u x[U]^T wuv
            xagg_ps = ps.tile([P, KC * v1n * H1], f32, tag="xagg")
            for c in range(KC):
                for v in range(v1n):
                    nc.tensor.matmul(
                        out=xagg_ps[:, (c * v1n + v) * H1:
                                    (c * v1n + v + 1) * H1],
                        lhsT=xu_sb[:, c * P:(c + 1) * P],
                        rhs=wuv_sb[v][:],
                        start=True, stop=True)
            xagg8 = sb.tile([P, KC * v1n * H1], fp8, tag="xagg8")
            nc.vector.tensor_copy(out=xagg8[:], in_=xagg_ps[:])
            xagg8_v = xagg8[:].rearrange("p (c v h) -> p c v h", c=KC, v=v1n)

            # ---- the big GEMM: agg[f, (h,fc,v)] = xagg @ (64*W1)
            # one accumulate pass per W1 piece as its DMA lands;
            # fp8 x fp8 -> f32 PSUM.  W1 block k = columns [k*128,(k+1)*128)
            # (k = h*KC + fc), so lhsT slices are contiguous per piece.
            # one start=True matmul zeroes the whole bank (the PSUM zero
            # region is 2KB-coarse, so per-block starts would wipe
            # neighbors); every accumulating matmul then uses start=False.
            agg_ps = ps.tile([P, nblk * v1n], f32, tag="agg")
            zrow = cpool.tile([1, P], bf16, tag="zrow")
            nc.vector.memset(zrow[:], 0.0)
            zcols = cpool.tile([1, nblk * v1n], bf16, tag="zcols")
            nc.vector.memset(zcols[:], 0.0)
            nc.tensor.matmul(out=agg_ps[:], lhsT=zrow[:], rhs=zcols[:],
                             start=True, stop=False, skip_group_check=True)
            for i, (c, k0, k1) in enumerate(W1_PIECES):
                for k in range(k0, k1):
                    h = k // KC
                    nc.tensor.matmul(
                        out=agg_ps[:, k * v1n:(k + 1) * v1n],
                        lhsT=w1_sb[i][:, (k - k0) * P:(k - k0 + 1) * P],
                        rhs=xagg8_v[:, c, :, h],
                        start=False, stop=(c == KC - 1),
                        skip_group_check=True)

            # elu'(x) = elu(x) + 1 = max(x,0) + min(exp(x),1); x = agg/64.
            # The -1 is folded into host constants downstream.  Computed in
            # block ranges matching the W1 piece splits so only the last 8
            # blocks' elu trails the final DMA.
            t1_sb = sb.tile([P, nblk * v1n], f32, tag="t1_sb")
            ee2_sb = sb.tile([P, nblk * v1n], f32, tag="ee2_sb")
            helu_sb = sb.tile([P, nblk * v1n], bf16, tag="helu_sb")
            for (k0, k1) in [(0, 24), (24, 40), (40, nblk)]:
                cl = slice(k0 * v1n, k1 * v1n)
                nc.vector.tensor_scalar(out=t1_sb[:, cl], in0=agg_ps[:, cl],
                                        scalar1=1.0 / W1SCALE, scalar2=0.0,
                                        op0=mybir.AluOpType.mult,
                                        op1=mybir.AluOpType.max)
                nc.scalar.activation(out=ee2_sb[:, cl], in_=agg_ps[:, cl],
                                     func=mybir.ActivationFunctionType.Exp,
                                     scale=1.0 / W1SCALE)
                nc.vector.tensor_scalar(out=ee2_sb[:, cl], in0=ee2_sb[:, cl],
                                        scalar1=1.0, scalar2=None,
                                        op0=mybir.AluOpType.min)
                nc.vector.tensor_tensor(out=helu_sb[:, cl], in0=t1_sb[:, cl],
                                        in1=ee2_sb[:, cl],
                                        op=mybir.AluOpType.add)

            # ---- oxm = x[m] @ wf_bot + bias3s (off critical path) ----
            xm_v = cv32("xm")
            wfb_v = cv32("wfb").rearrange("p (k n) -> p k n", k=KC)
            one11_v = cv32("one11")
            bias3s_v = cv32("bias3s")
            tailb = ps.tile([P, 12], f32, tag="tailbank")
            oxm_ps = tailb[0:1, 0:2]
            h2f_ps = tailb[0:v1n, 2:6]
            r2t_ps = tailb[0:s1n, 6:7]
            den_ps = tailb[0:1, 7:8]
            fin_ps = tailb[0:1, 8:10]
            for c in range(KC):
                nc.tensor.matmul(out=oxm_ps, lhsT=xm_v[:, c:c + 1],
                                 rhs=wfb_v[:, c, :],
                                 start=(c == 0), stop=False)
            nc.tensor.matmul(out=oxm_ps, lhsT=one11_v, rhs=bias3s_v,
                             start=False, stop=True)

            # ---- folded layer-2: h2f'[v, 0:4] = helu' @ [w2fold|Ws2|Wd2]
            for k in range(nblk):
                nc.tensor.matmul(out=h2f_ps,
                                 lhsT=helu_sb[:, k * v1n:(k + 1) * v1n],
                                 rhs=w2f_sb[:, k * 4:(k + 1) * 4],
                                 start=(k == 0), stop=(k == nblk - 1))
            h2f_sb = sb.tile([v1n, 4], f32, tag="h2f_sb")
            nc.vector.tensor_copy(out=h2f_sb[:], in_=h2f_ps)

            # ---- layer-2 logits (transposed), softmax, weighted sum ----
            g_v = cv32("g")
            gm_v = cv32("gm")
            shiftrow_v = cv32("shiftrow")
            ones_s1_v = cv32("ones_s1")
            nc.tensor.matmul(out=r2t_ps, lhsT=g_v, rhs=h2f_sb[:, 2:3],
                             start=True, stop=False)
            nc.tensor.matmul(out=r2t_ps, lhsT=gm_v, rhs=h2f_sb[:, 3:4],
                             start=False, stop=False)
            nc.tensor.matmul(out=r2t_ps, lhsT=shiftrow_v, rhs=one11_v,
                             start=False, stop=True)
            # exp(lrelu(x)) = max(exp(x), exp(0.2x)) -- two Act ops
            # back-to-back on one engine beat DVE/Act ping-pong here.
            ea_sb = sb.tile([s1n, 1], f32, tag="ea_sb")
            nc.scalar.activation(out=ea_sb[:], in_=r2t_ps,
                                 func=mybir.ActivationFunctionType.Exp)
            eb_sb = sb.tile([s1n, 1], f32, tag="eb_sb")
            nc.scalar.activation(out=eb_sb[:], in_=r2t_ps,
                                 func=mybir.ActivationFunctionType.Exp,
                                 scale=0.2)
            e2t_sb = sb.tile([s1n, 1], f32, tag="e2t_sb")
            nc.vector.tensor_tensor(out=e2t_sb[:], in0=ea_sb[:],
                                    in1=eb_sb[:], op=mybir.AluOpType.max)

            nc.tensor.matmul(out=den_ps, lhsT=e2t_sb[:], rhs=ones_s1_v,
                             start=True, stop=True)
            if s1_ident:
                nc.tensor.matmul(out=fin_ps, lhsT=e2t_sb[:],
                                 rhs=h2f_sb[:, 0:2], start=True, stop=True)
            else:
                gath_ps = tailb[0:s1n, 10:12]
                nc.tensor.matmul(out=gath_ps, lhsT=g_v,
                                 rhs=h2f_sb[:, 0:2], start=True, stop=True)
                gath_sb = sb.tile([s1n, 2], f32, tag="gath_sb")
                nc.vector.tensor_copy(out=gath_sb[:], in_=gath_ps)
                nc.tensor.matmul(out=fin_ps, lhsT=e2t_sb[:],
                                 rhs=gath_sb[:], start=True, stop=True)

            rec2 = sb.tile([1, 1], f32, tag="rec2")
            nc.vector.reciprocal(out=rec2[:], in_=den_ps)
            res1 = sb.tile([1, 2], f32, tag="res1")
            nc.vector.tensor_scalar(out=res1[:], in0=fin_ps,
                                    scalar1=rec2[:, 0:1], scalar2=None,
                                    op0=mybir.AluOpType.mult)
            res_sb = sb.tile([1, 2], f32, tag="res_sb")
            nc.vector.tensor_add(out=res_sb[:], in0=res1[:],
                                 in1=oxm_ps)
            nc.sync.dma_start(out=d_res[:], in_=res_sb[:])

    nc.compile()
    return nc


_CACHE = {}


def _get_nc(meta):
    key = repr(sorted(meta.items()))
    if key not in _CACHE:
        _CACHE[key] = _build(meta)
    return _CACHE[key]


def make_in_maps(**inputs):
    x = np.asarray(inputs["x"], np.float32)
    n_nodes = x.shape[0]
    meta, host = _preprocess(inputs["edge_index"], inputs["mask_idx"], n_nodes)
    v1n, s1n, up = meta["v1n"], meta["s1n"], meta["up"]
    s2p = v1n * meta["gmax"]
    nblk = H1 * KC

    W1 = np.asarray(inputs["W1"], np.float32)
    att_s1 = np.asarray(inputs["att_src1"], np.float32)
    att_d1 = np.asarray(inputs["att_dst1"], np.float32)
    b1 = np.asarray(inputs["b1"], np.float32)
    W2 = np.asarray(inputs["W2"], np.float32)
    att_s2 = np.asarray(inputs["att_src2"], np.float32)
    att_d2 = np.asarray(inputs["att_dst2"], np.float32)
    b2 = np.asarray(inputs["b2"], np.float32)
    fc_w = np.asarray(inputs["fc_w"], np.float32)
    fc_b = np.asarray(inputs["fc_b"], np.float32)
    cls_w = np.asarray(inputs["cls_w"], np.float32)
    cls_b = np.asarray(inputs["cls_b"], np.float32)

    # weight-weight folds
    Ws1 = np.einsum("chf,hf->ch", W1.reshape(C, H1, OUT), att_s1)   # [C, H1]
    Wd1 = np.einsum("chf,hf->ch", W1.reshape(C, H1, OUT), att_d1)
    Ws2 = W2 @ att_s2[0]                                            # [H1*OUT]
    Wd2 = W2 @ att_d2[0]
    wf = fc_w @ cls_w                                               # [1536, 2]
    wf_top, wf_bot = wf[:OUT], wf[OUT:]
    w2fold = W2 @ wf_top                                            # [6144, 2]
    # helu' = elu + 1 fold: subtract column sums; softmax shift constant
    shift_const = -(Ws2.sum() + Wd2.sum())
    bias3s = (b2 @ wf_top + fc_b @ cls_w + cls_b
              - w2fold.sum(axis=0)).reshape(1, 2).astype(np.float32)

    # w2f blocks ordered to match agg blocks k = h*KC + fc
    w2f4 = np.concatenate([w2fold, Ws2[:, None], Wd2[:, None]], axis=1)
    w2f_host = np.zeros((P, nblk * 4), np.float32)
    for k in range(nblk):
        w2f_host[:, k * 4:(k + 1) * 4] = w2f4[k * P:(k + 1) * P, :]

    # bf16 constants tensor
    lay16, cw16 = _lay16(meta)
    cst16 = np.zeros((P, cw16), np.float32)

    def fill16(name, arr):
        rows, off, cols = lay16[name]
        assert arr.shape == (rows, cols), (name, arr.shape, (rows, cols))
        cst16[0:rows, off:off + cols] = arr

    xu_rows = np.zeros((up, C), np.float32)
    xu_rows[:meta["un"]] = x[host["u"]]
    xut = np.zeros((P, KC * up), np.float32)
    for c in range(KC):
        xut[:, c * up:(c + 1) * up] = xu_rows[:, c * P:(c + 1) * P].T
    fill16("xut", xut)
    fill16("wsd1", _chunked(np.concatenate([Ws1, Wd1], axis=1)))
    fill16("u2e", host["u2e"])
    fill16("d2e", host["d2e"])
    fill16("pad01", host["pad01"])
    fill16("neg8", np.full((1, H1), NEGPAD, np.float32))
    fill16("sv01", host["sv01"])

    # f32 constants tensor (tail)
    lay32, cw32 = _lay32(meta)
    cst32 = np.zeros((P, cw32), np.float32)

    def fill32(name, arr):
        rows, off, cols = lay32[name]
        assert arr.shape == (rows, cols), (name, arr.shape, (rows, cols))
        cst32[0:rows, off:off + cols] = arr

    fill32("xm", np.ascontiguousarray(x[host["m"]].reshape(KC, P).T))
    fill32("wfb", _chunked(np.ascontiguousarray(wf_bot)))
    fill32("g", host["g"])
    fill32("gm", host["gm"])
    fill32("shiftrow", np.full((1, s1n), shift_const, np.float32))
    fill32("one11", np.ones((1, 1), np.float32))
    fill32("bias3s", bias3s)
    fill32("ones_s1", np.ones((s1n, 1), np.float32))

    assert not np.any(b1), "b1 != 0 not supported by this build"
    w1s = (W1 * W1SCALE).astype(np_fp8)                 # [768, 6144] fp8

    im = {
        "cst16": cst16.astype(np_bf16),
        "xu": xu_rows.astype(np_bf16),
        "w2f": w2f_host.astype(np_bf16),
        "cst32": cst32,
    }
    pieces = [(c, 0, nblk) for c in range(KC - 1)] + [
        (KC - 1, 0, 24), (KC - 1, 24, 40), (KC - 1, 40, nblk)]
    for i, (c, k0, k1) in enumerate(pieces):
        im[f"w1p{i}"] = np.ascontiguousarray(
            w1s[c * P:(c + 1) * P, k0 * P:k1 * P])
    return meta, [im] * NCORES


def kernel(**inputs):
    meta, in_maps = make_in_maps(**inputs)
    nc = _get_nc(meta)
    res = run_bass_kernel_spmd(nc, in_maps, core_ids=list(range(NCORES)))
    return res.results[0]["res"].astype(np.float32)


# revision 16
# speedup vs baseline: 1.1218x; 1.0801x over previous
"""Trainium2 Bass kernel for the 2-layer GAT node-classification head.

The reference reads only h2[mask_idx] and x[mask_idx] for the classifier, so
the exact computation collapses to mask_idx's 2-hop in-neighborhood:

  V1 = sources of mask's in-edges (incl. the self-loop), S2 = in-edges of V1,
  U  = unique sources of S2.  |V1|=2, |S2|=7, |U|=6 for this graph.

Per-core plan (identical on all 8 cores -- the cost model charges a flat
15us constant for ANY collective, which dwarfs the whole problem, so the
fastest distribution is full replication with zero communication):

  1. attention: a_src/a_dst at U via folded Ws1/Wd1 (one-hot scatter to the
     edge layout), segment softmax without max-shift (logits are tiny), all
     heads at once.
  2. aggregate-first: since the value aggregation is linear in x, build
     per-(head, dst) weighted x sums (xagg) BEFORE the big GEMM; the
     [768 x 6144] W1 GEMM then has only v1n output columns per head.
  3. W1 streams in fp8 (x64 prescale to clear the e4m3 subnormal range) in
     6 chunk DMAs pipelined against the PSUM-accumulating GEMM.  DMA bytes
     dominate the kernel; fp8 quarters them vs f32.
  4. elu via exp(min(x,0)) = min(exp(x),1); the "-1" of elu folds into host
     constants.  Layer-2 logits/softmax and the classifier fold into a
     [6144, 4] bf16 contraction + tiny fixed tail.

Host preprocessing: graph cone extraction + one-hot scatter matrices
(index-select = sharding) and weight-weight folds (W1@att, W2@fold), as in
the original head-sharded version.
"""

import numpy as np
import ml_dtypes

import concourse.bass as bass
import concourse.mybir as mybir
import concourse.tile as tile
from concourse import bacc
from concourse.bass_utils import run_bass_kernel_spmd
from concourse.masks import make_identity

NCORES = 8
P = 128
C = 768          # input feature dim
H1 = 8           # layer-1 heads
OUT = 768        # per-head feature dim
KC = C // P      # 6 k-chunks of 128 over the 768 contraction
NEGPAD = -745.0  # padding logit: exp(0.2 * NEGPAD) == 0 in f32
W1SCALE = 64.0   # fp8 prescale for W1 (clears e4m3 subnormals)

f32 = mybir.dt.float32
bf16 = mybir.dt.bfloat16
fp8 = mybir.dt.float8e4
np_bf16 = ml_dtypes.bfloat16
np_fp8 = ml_dtypes.float8_e4m3


# ---------------------------------------------------------------- host graph
def _preprocess(edge_index, mask_idx, n_nodes):
    """Extract the 2-hop in-neighborhood of mask_idx. meta is compile-time
    (shapes only); host holds the data (one-hot matrices, index lists)."""
    ei = np.asarray(edge_index).astype(np.int64)
    m = int(np.asarray(mask_idx))
    src_all = np.concatenate([ei[0], np.arange(n_nodes, dtype=np.int64)])
    dst_all = np.concatenate([ei[1], np.arange(n_nodes, dtype=np.int64)])

    s1_pos = np.nonzero(dst_all == m)[0]          # in-edges of m (incl self)
    s1_src = src_all[s1_pos].tolist()
    s1n = len(s1_src)
    v1 = list(dict.fromkeys(s1_src))              # unique sources
    v1n = len(v1)
    assert v1n <= 8, f"mask in-degree too large for this layout: {v1n}"

    groups = [src_all[np.nonzero(dst_all == v)[0]].tolist() for v in v1]
    gmax = max(len(g) for g in groups)
    s2p = v1n * gmax
    assert s2p <= P, f"edge tile too large: {s2p}"

    u = list(dict.fromkeys([s for g in groups for s in g]))
    un = len(u)
    up = 16
    while up < un:
        up *= 2
    assert v1n * up <= P, f"wuv tile too large: {v1n * up}"
    urow = {node: r for r, node in enumerate(u)}

    # S2 edge slot layout: group g occupies cols [g*gmax, g*gmax+len(g))
    u2e = np.zeros((up, s2p), np.float32)         # src scatter
    d2e = np.zeros((up, s2p), np.float32)         # dst scatter
    pad01 = np.zeros((1, s2p), np.float32)
    sv01 = np.zeros((s2p, v1n * up), np.float32)  # edge -> (v,u) accumulate
    for g, srcs in enumerate(groups):
        for j in range(gmax):
            e = g * gmax + j
            if j < len(srcs):
                su = urow[srcs[j]]
                u2e[su, e] = 1.0
                d2e[urow[v1[g]], e] = 1.0
                sv01[e, g * up + su] = 1.0
            else:
                pad01[0, e] = 1.0

    # layer-2 (s1) structure
    v1row = {v: r for r, v in enumerate(v1)}
    g_mat = np.zeros((v1n, s1n), np.float32)
    gm_mat = np.zeros((v1n, s1n), np.float32)
    for e, s in enumerate(s1_src):
        g_mat[v1row[s], e] = 1.0
        gm_mat[v1row[m], e] = 1.0
    s1_ident = (s1n == v1n) and all(v1row[s] == e for e, s in enumerate(s1_src))

    meta = dict(v1n=v1n, s1n=s1n, gmax=gmax, un=un, up=up, s1_ident=s1_ident)
    host = dict(m=m, v1=v1, u=u, u2e=u2e, d2e=d2e, pad01=pad01, sv01=sv01,
                g=g_mat, gm=gm_mat)
    return meta, host


def _lay16(meta):
    """Column layout of the bf16 packed-constants tensor."""
    up, s2p = meta["up"], meta["v1n"] * meta["gmax"]
    pieces = [
        ("xut", P, KC * up),        # x[U]^T chunked  [128, KC*up]
        ("wsd1", P, KC * 2 * H1),   # [Ws1|Wd1] chunked
        ("u2e", up, s2p),
        ("d2e", up, s2p),
        ("pad01", 1, s2p),
        ("neg8", 1, H1),
        ("sv01", s2p, meta["v1n"] * up),
    ]
    lay, off = {}, 0
    for name, rows, cols in pieces:
        lay[name] = (rows, off, cols)
        off += cols
    return lay, off


def _lay32(meta):
    """Column layout of the f32 packed-constants tensor (tail/oxm)."""
    v1n, s1n = meta["v1n"], meta["s1n"]
    pieces = [
        ("xm", P, KC),
        ("wfb", P, KC * 2),
        ("g", v1n, s1n),
        ("gm", v1n, s1n),
        ("shiftrow", 1, s1n),
        ("one11", 1, 1),
        ("bias3s", 1, 2),
        ("ones_s1", s1n, 1),
    ]
    lay, off = {}, 0
    for name, rows, cols in pieces:
        lay[name] = (rows, off, cols)
        off += cols
    return lay, off


def _chunked(w):
    """[K, N] -> [128, (K//128)*N] chunk-major free layout."""
    k, n = w.shape
    assert k % P == 0
    return np.ascontiguousarray(
        w.reshape(k // P, P, n).transpose(1, 0, 2).reshape(P, (k // P) * n))


# ---------------------------------------------------------------- bass build
def _build(meta):
    v1n, s1n, gmax = meta["v1n"], meta["s1n"], meta["gmax"]
    up, s1_ident = meta["up"], meta["s1_ident"]
    s2p = v1n * gmax
    nblk = H1 * KC                  # 48 (head, f-chunk) output blocks
    lay16, cw16 = _lay16(meta)
    lay32, cw32 = _lay32(meta)

    nc = bacc.Bacc("TRN2", target_bir_lowering=False, debug=False,
                   enable_asserts=True, num_devices=NCORES)

    d_cst16 = nc.dram_tensor("cst16", [P, cw16], bf16, kind="ExternalInput")
    d_xu = nc.dram_tensor("xu", [up, C], bf16, kind="ExternalInput")
    # W1 stream pieces: full chunks c0..c4, then chunk 5 split into graded
    # block groups so only 8 blocks' matmuls + a small elu slice trail the
    # final DMA semaphore.
    W1_PIECES = [(c, 0, nblk) for c in range(KC - 1)] + [
        (KC - 1, 0, 24), (KC - 1, 24, 40), (KC - 1, 40, nblk)]
    d_w1 = [nc.dram_tensor(f"w1p{i}", [P, (k1 - k0) * P], fp8,
                           kind="ExternalInput")
            for i, (c, k0, k1) in enumerate(W1_PIECES)]
    d_w2f = nc.dram_tensor("w2f", [P, nblk * 4], bf16, kind="ExternalInput")
    d_cst32 = nc.dram_tensor("cst32", [P, cw32], f32, kind="ExternalInput")
    d_res = nc.dram_tensor("res", [1, 2], f32, kind="ExternalOutput")

    with tile.TileContext(nc) as tc:
        with (
            tc.tile_pool(name="const", bufs=1) as cpool,
            tc.tile_pool(name="sbuf", bufs=1) as sb,
            tc.tile_pool(name="big", bufs=1) as bigp,
            tc.tile_pool(name="ps", bufs=1, space="PSUM") as ps,
        ):
            # ---- input DMAs (all SP-issued: the SP sequencer serializes
            # issue order, keeping the W1 stream contiguous on the wire).
            # w1p0 first: its transfer hides the HWDGE generation of the
            # small attention tensors.
            w1_sb = [bigp.tile([P, (k1 - k0) * P], fp8, tag=f"w1_{i}",
                               name=f"w1_{i}")
                     for i, (c, k0, k1) in enumerate(W1_PIECES)]
            nc.sync.dma_start(out=w1_sb[0][:], in_=d_w1[0][:])
            cst16 = cpool.tile([P, cw16], bf16, tag="cst16")
            nc.sync.dma_start(out=cst16[:], in_=d_cst16[:])
            xu_sb = cpool.tile([up, C], bf16, tag="xu")
            nc.sync.dma_start(out=xu_sb[:], in_=d_xu[:])
            for i in range(1, len(W1_PIECES)):
                nc.sync.dma_start(out=w1_sb[i][:], in_=d_w1[i][:])
            w2f_sb = cpool.tile([P, nblk * 4], bf16, tag="w2f")
            nc.sync.dma_start(out=w2f_sb[:], in_=d_w2f[:])
            cst32 = cpool.tile([P, cw32], f32, tag="cst32")
            nc.sync.dma_start(out=cst32[:], in_=d_cst32[:])

            def cv16(name):
                rows, off, cols = lay16[name]
                return cst16[0:rows, off:off + cols]

            def cv32(name):
                rows, off, cols = lay32[name]
                return cst32[0:rows, off:off + cols]

            xut_v = cv16("xut").rearrange("p (k n) -> p k n", k=KC)
            wsd1_v = cv16("wsd1").rearrange("p (k n) -> p k n", k=KC)
            u2e_v = cv16("u2e")
            d2e_v = cv16("d2e")
            pad01_v = cv16("pad01")
            neg8_v = cv16("neg8")
            sv01_v = cv16("sv01")

            ident = cpool.tile([H1, H1], f32, tag="ident")
            make_identity(nc, ident[:])

            # ---- attention: a_src/a_dst at U, all heads ----
            attb = ps.tile([P, 512], f32, tag="attbank")
            asd_ps = attb[0:up, 0:2 * H1]
            lg_ps = attb[0:H1, 16:16 + s2p]
            at_ps = attb[0:s2p, 144:144 + H1]
            wuv_ps = [attb[0:up, 152 + 8 * v:160 + 8 * v]
                      for v in range(v1n)]
            for c in range(KC):
                nc.tensor.matmul(out=asd_ps, lhsT=xut_v[:, c, :],
                                 rhs=wsd1_v[:, c, :],
                                 start=(c == 0), stop=(c == KC - 1))
            asd_sb = sb.tile([up, 2 * H1], bf16, tag="asd_sb")
            nc.vector.tensor_copy(out=asd_sb[:], in_=asd_ps)

            # per-edge logits: a_s[src_e] + a_d[dst_e] + pad bias
            nc.tensor.matmul(out=lg_ps, lhsT=asd_sb[:, 0:H1], rhs=u2e_v,
                             start=True, stop=False)
            nc.tensor.matmul(out=lg_ps, lhsT=asd_sb[:, H1:2 * H1],
                             rhs=d2e_v, start=False, stop=False)
            nc.tensor.matmul(out=lg_ps, lhsT=neg8_v, rhs=pad01_v,
                             start=False, stop=True)

            # leaky-relu (one Act op), exp without max-shift (logits tiny),
            # then per-group normalize
            lg_t = sb.tile([H1, s2p], f32, tag="lg_t")
            nc.vector.tensor_scalar_mul(out=lg_t[:], in0=lg_ps, scalar1=0.2)
            lg_sb = sb.tile([H1, s2p], f32, tag="lg_sb")
            nc.vector.tensor_tensor(out=lg_sb[:], in0=lg_ps, in1=lg_t[:],
                                    op=mybir.AluOpType.max)
            ee_sb = sb.tile([H1, s2p], f32, tag="ee_sb")
            nc.scalar.activation(out=ee_sb[:], in_=lg_sb[:],
                                 func=mybir.ActivationFunctionType.Exp)
            eev = ee_sb[:].rearrange("h (g e) -> h g e", e=gmax)
            den = sb.tile([H1, v1n], f32, tag="den")
            nc.vector.reduce_sum(out=den[:], in_=eev,
                                 axis=mybir.AxisListType.X)
            rec = sb.tile([H1, v1n], f32, tag="rec")
            nc.vector.reciprocal(out=rec[:], in_=den[:])
            alpha_sb = sb.tile([H1, s2p], f32, tag="alpha_sb")
            recb = rec[:].rearrange("h (g o) -> h g o", o=1).to_broadcast(
                [H1, v1n, gmax])
            nc.vector.tensor_tensor(
                out=alpha_sb[:].rearrange("h (g e) -> h g e", e=gmax),
                in0=eev, in1=recb, op=mybir.AluOpType.mult)

            # alpha^T via PE transpose, then wuv[(v,u), h] = sum_e alpha
            nc.tensor.transpose(out=at_ps, in_=alpha_sb[:],
                                identity=ident[:])
            at_sb = sb.tile([s2p, H1], bf16, tag="at_sb")
            nc.vector.tensor_copy(out=at_sb[:], in_=at_ps)
            # per-v blocks: PE/DVE partition bases must be 0/32/64-aligned
            wuv_sb = [sb.tile([up, H1], bf16, tag=f"wuv_sb{v}",
                              name=f"wuv_sb{v}") for v in range(v1n)]
            for v in range(v1n):
                nc.tensor.matmul(out=wuv_ps[v],
                                 lhsT=sv01_v[:, v * up:(v + 1) * up],
                                 rhs=at_sb[:], start=True, stop=True)
                nc.vector.tensor_copy(out=wuv_sb[v][:], in_=wuv_ps[v])

            # xagg^T chunks: [128c, (c,v,h)] = sum_u x[U]^T wuv
            xagg_ps = ps.tile([P, KC * v1n * H1], f32, tag="xagg")
            for c in range(KC):
                for v in range(v1n):
                    nc.tensor.matmul(
                        out=xagg_ps[:, (c * v1n + v) * H1:
                                    (c * v1n + v + 1) * H1],
                        lhsT=xu_sb[:, c * P:(c + 1) * P],
                        rhs=wuv_sb[v][:],
                        start=True, stop=True)
            xagg8 = sb.tile([P, KC * v1n * H1], fp8, tag="xagg8")
            nc.vector.tensor_copy(out=xagg8[:], in_=xagg_ps[:])
            xagg8_v = xagg8[:].rearrange("p (c v h) -> p c v h", c=KC, v=v1n)

            # ---- the big GEMM: agg[f, (h,fc,v)] = xagg @ (64*W1)
            # one accumulate pass per W1 piece as its DMA lands;
            # fp8 x fp8 -> f32 PSUM.  W1 block k = columns [k*128,(k+1)*128)
            # (k = h*KC + fc), so lhsT slices are contiguous per piece.
            # one start=True matmul zeroes the whole bank (the PSUM zero
            # region is 2KB-coarse, so per-block starts would wipe
            # neighbors); every accumulating matmul then uses start=False.
            agg_ps = ps.tile([P, nblk * v1n], f32, tag="agg")
            zrow = cpool.tile([1, P], bf16, tag="zrow")
            nc.vector.memset(zrow[:], 0.0)
            zcols = cpool.tile([1, nblk * v1n], bf16, tag="zcols")
            nc.vector.memset(zcols[:], 0.0)
            nc.tensor.matmul(out=agg_ps[:], lhsT=zrow[:], rhs=zcols[:],
                             start=True, stop=False, skip_group_check=True)
            for i, (c, k0, k1) in enumerate(W1_PIECES):
                for k in range(k0, k1):
                    h = k // KC
                    nc.tensor.matmul(
                        out=agg_ps[:, k * v1n:(k + 1) * v1n],
                        lhsT=w1_sb[i][:, (k - k0) * P:(k - k0 + 1) * P],
                        rhs=xagg8_v[:, c, :, h],
                        start=False, stop=(c == KC - 1),
                        skip_group_check=True)

            # elu'(x) = elu(x) + 1 = max(x,0) + min(exp(x),1); x = agg/64.
            # The -1 is folded into host constants downstream.  Computed in
            # block ranges matching the W1 piece splits so only the last 8
            # blocks' elu trails the final DMA.
            t1_sb = sb.tile([P, nblk * v1n], f32, tag="t1_sb")
            ee2_sb = sb.tile([P, nblk * v1n], f32, tag="ee2_sb")
            helu_sb = sb.tile([P, nblk * v1n], bf16, tag="helu_sb")
            nc.vector.tensor_scalar(out=t1_sb[:], in0=agg_ps[:],
                                    scalar1=1.0 / W1SCALE, scalar2=0.0,
                                    op0=mybir.AluOpType.mult,
                                    op1=mybir.AluOpType.max)
            nc.scalar.activation(out=ee2_sb[:], in_=agg_ps[:],
                                 func=mybir.ActivationFunctionType.Exp,
                                 scale=1.0 / W1SCALE)
            nc.vector.tensor_scalar(out=ee2_sb[:], in0=ee2_sb[:],
                                    scalar1=1.0, scalar2=None,
                                    op0=mybir.AluOpType.min)
            nc.vector.tensor_tensor(out=helu_sb[:], in0=t1_sb[:],
                                    in1=ee2_sb[:], op=mybir.AluOpType.add)

            # ---- oxm = x[m] @ wf_bot + bias3s (off critical path) ----
            xm_v = cv32("xm")
            wfb_v = cv32("wfb").rearrange("p (k n) -> p k n", k=KC)
            one11_v = cv32("one11")
            bias3s_v = cv32("bias3s")
            tailb = ps.tile([P, 12], f32, tag="tailbank")
            oxm_ps = tailb[0:1, 0:2]
            h2f_ps = tailb[0:v1n, 2:6]
            r2t_ps = tailb[0:s1n, 6:7]
            den_ps = tailb[0:1, 7:8]
            fin_ps = tailb[0:1, 8:10]
            for c in range(KC):
                nc.tensor.matmul(out=oxm_ps, lhsT=xm_v[:, c:c + 1],
                                 rhs=wfb_v[:, c, :],
                                 start=(c == 0), stop=False)
            nc.tensor.matmul(out=oxm_ps, lhsT=one11_v, rhs=bias3s_v,
                             start=False, stop=True)

            # ---- folded layer-2: h2f'[v, 0:4] = helu' @ [w2fold|Ws2|Wd2]
            for k in range(nblk):
                nc.tensor.matmul(out=h2f_ps,
                                 lhsT=helu_sb[:, k * v1n:(k + 1) * v1n],
                                 rhs=w2f_sb[:, k * 4:(k + 1) * 4],
                                 start=(k == 0), stop=(k == nblk - 1))
            h2f_sb = sb.tile([v1n, 4], f32, tag="h2f_sb")
            nc.vector.tensor_copy(out=h2f_sb[:], in_=h2f_ps)

            # ---- layer-2 logits (transposed), softmax, weighted sum ----
            g_v = cv32("g")
            gm_v = cv32("gm")
            shiftrow_v = cv32("shiftrow")
            ones_s1_v = cv32("ones_s1")
            nc.tensor.matmul(out=r2t_ps, lhsT=g_v, rhs=h2f_sb[:, 2:3],
                             start=True, stop=False)
            nc.tensor.matmul(out=r2t_ps, lhsT=gm_v, rhs=h2f_sb[:, 3:4],
                             start=False, stop=False)
            nc.tensor.matmul(out=r2t_ps, lhsT=shiftrow_v, rhs=one11_v,
                             start=False, stop=True)
            # exp(lrelu(x)) = max(exp(x), exp(0.2x)) -- two Act ops
            # back-to-back on one engine beat DVE/Act ping-pong here.
            ea_sb = sb.tile([s1n, 1], f32, tag="ea_sb")
            nc.scalar.activation(out=ea_sb[:], in_=r2t_ps,
                                 func=mybir.ActivationFunctionType.Exp)
            eb_sb = sb.tile([s1n, 1], f32, tag="eb_sb")
            nc.scalar.activation(out=eb_sb[:], in_=r2t_ps,
                                 func=mybir.ActivationFunctionType.Exp,
                                 scale=0.2)
            e2t_sb = sb.tile([s1n, 1], f32, tag="e2t_sb")
            nc.vector.tensor_tensor(out=e2t_sb[:], in0=ea_sb[:],
                                    in1=eb_sb[:], op=mybir.AluOpType.max)

            nc.tensor.matmul(out=den_ps, lhsT=e2t_sb[:], rhs=ones_s1_v,
                             start=True, stop=True)
            if s1_ident:
                nc.tensor.matmul(out=fin_ps, lhsT=e2t_sb[:],
                                 rhs=h2f_sb[:, 0:2], start=True, stop=True)
            else:
                gath_ps = tailb[0:s1n, 10:12]
                nc.tensor.matmul(out=gath_ps, lhsT=g_v,
                                 rhs=h2f_sb[:, 0:2], start=True, stop=True)
                gath_sb = sb.tile([s1n, 2], f32, tag="gath_sb")
                nc.vector.tensor_copy(out=gath_sb[:], in_=gath_ps)
                nc.tensor.matmul(out=fin_ps, lhsT=e2t_sb[:],
                                 rhs=gath_sb[:], start=True, stop=True)

            rec2 = sb.tile([1, 1], f32, tag="rec2")
            nc.vector.reciprocal(out=rec2[:], in_=den_ps)
            res1 = sb.tile([1, 2], f32, tag="res1")
            nc.vector.tensor_scalar(out=res1[:], in0=fin_ps,
                                    scalar1=rec2[:, 0:1], scalar2=None,
                                    op0=mybir.AluOpType.mult)
            res_sb = sb.tile([1, 2], f32, tag="res_sb")
            nc.vector.tensor_add(out=res_sb[:], in0=res1[:],
                                 in1=oxm_ps)
            nc.sync.dma_start(out=d_res[:], in_=res_sb[:])

    nc.compile()
    return nc


_CACHE = {}


def _get_nc(meta):
    key = repr(sorted(meta.items()))
    if key not in _CACHE:
        _CACHE[key] = _build(meta)
    return _CACHE[key]


def make_in_maps(**inputs):
    x = np.asarray(inputs["x"], np.float32)
    n_nodes = x.shape[0]
    meta, host = _preprocess(inputs["edge_index"], inputs["mask_idx"], n_nodes)
    v1n, s1n, up = meta["v1n"], meta["s1n"], meta["up"]
    s2p = v1n * meta["gmax"]
    nblk = H1 * KC

    W1 = np.asarray(inputs["W1"], np.float32)
    att_s1 = np.asarray(inputs["att_src1"], np.float32)
    att_d1 = np.asarray(inputs["att_dst1"], np.float32)
    b1 = np.asarray(inputs["b1"], np.float32)
    W2 = np.asarray(inputs["W2"], np.float32)
    att_s2 = np.asarray(inputs["att_src2"], np.float32)
    att_d2 = np.asarray(inputs["att_dst2"], np.float32)
    b2 = np.asarray(inputs["b2"], np.float32)
    fc_w = np.asarray(inputs["fc_w"], np.float32)
    fc_b = np.asarray(inputs["fc_b"], np.float32)
    cls_w = np.asarray(inputs["cls_w"], np.float32)
    cls_b = np.asarray(inputs["cls_b"], np.float32)

    # weight-weight folds
    Ws1 = np.einsum("chf,hf->ch", W1.reshape(C, H1, OUT), att_s1)   # [C, H1]
    Wd1 = np.einsum("chf,hf->ch", W1.reshape(C, H1, OUT), att_d1)
    Ws2 = W2 @ att_s2[0]                                            # [H1*OUT]
    Wd2 = W2 @ att_d2[0]
    wf = fc_w @ cls_w                                               # [1536, 2]
    wf_top, wf_bot = wf[:OUT], wf[OUT:]
    w2fold = W2 @ wf_top                                            # [6144, 2]
    # helu' = elu + 1 fold: subtract column sums; softmax shift constant
    shift_const = -(Ws2.sum() + Wd2.sum())
    bias3s = (b2 @ wf_top + fc_b @ cls_w + cls_b
              - w2fold.sum(axis=0)).reshape(1, 2).astype(np.float32)

    # w2f blocks ordered to match agg blocks k = h*KC + fc
    w2f4 = np.concatenate([w2fold, Ws2[:, None], Wd2[:, None]], axis=1)
    w2f_host = np.zeros((P, nblk * 4), np.float32)
    for k in range(nblk):
        w2f_host[:, k * 4:(k + 1) * 4] = w2f4[k * P:(k + 1) * P, :]

    # bf16 constants tensor
    lay16, cw16 = _lay16(meta)
    cst16 = np.zeros((P, cw16), np.float32)

    def fill16(name, arr):
        rows, off, cols = lay16[name]
        assert arr.shape == (rows, cols), (name, arr.shape, (rows, cols))
        cst16[0:rows, off:off + cols] = arr

    xu_rows = np.zeros((up, C), np.float32)
    xu_rows[:meta["un"]] = x[host["u"]]
    xut = np.zeros((P, KC * up), np.float32)
    for c in range(KC):
        xut[:, c * up:(c + 1) * up] = xu_rows[:, c * P:(c + 1) * P].T
    fill16("xut", xut)
    fill16("wsd1", _chunked(np.concatenate([Ws1, Wd1], axis=1)))
    fill16("u2e", host["u2e"])
    fill16("d2e", host["d2e"])
    fill16("pad01", host["pad01"])
    fill16("neg8", np.full((1, H1), NEGPAD, np.float32))
    fill16("sv01", host["sv01"])

    # f32 constants tensor (tail)
    lay32, cw32 = _lay32(meta)
    cst32 = np.zeros((P, cw32), np.float32)

    def fill32(name, arr):
        rows, off, cols = lay32[name]
        assert arr.shape == (rows, cols), (name, arr.shape, (rows, cols))
        cst32[0:rows, off:off + cols] = arr

    fill32("xm", np.ascontiguousarray(x[host["m"]].reshape(KC, P).T))
    fill32("wfb", _chunked(np.ascontiguousarray(wf_bot)))
    fill32("g", host["g"])
    fill32("gm", host["gm"])
    fill32("shiftrow", np.full((1, s1n), shift_const, np.float32))
    fill32("one11", np.ones((1, 1), np.float32))
    fill32("bias3s", bias3s)
    fill32("ones_s1", np.ones((s1n, 1), np.float32))

    assert not np.any(b1), "b1 != 0 not supported by this build"
    w1s = (W1 * W1SCALE).astype(np_fp8)                 # [768, 6144] fp8

    im = {
        "cst16": cst16.astype(np_bf16),
        "xu": xu_rows.astype(np_bf16),
        "w2f": w2f_host.astype(np_bf16),
        "cst32": cst32,
    }
    pieces = [(c, 0, nblk) for c in range(KC - 1)] + [
        (KC - 1, 0, 24), (KC - 1, 24, 40), (KC - 1, 40, nblk)]
    for i, (c, k0, k1) in enumerate(pieces):
        im[f"w1p{i}"] = np.ascontiguousarray(
            w1s[c * P:(c + 1) * P, k0 * P:k1 * P])
    return meta, [im] * NCORES


def kernel(**inputs):
    meta, in_maps = make_in_maps(**inputs)
    nc = _get_nc(meta)
    res = run_bass_kernel_spmd(nc, in_maps, core_ids=list(range(NCORES)))
    return res.results[0]["res"].astype(np.float32)
